# revision 1
# baseline (speedup 1.0000x reference)
"""Trainium2 Bass kernel for nn_AgentPolicy (sparse attention agent policy).

Data parallel over 8 NeuronCores: batch 4096 -> 512 per core.

Math (per batch row b; only comm[:, 0] of the reference attention is used):
  x  = relu(relu(obs@W1+b1)@W2+b2)                       [256]
  tok = [x, msg_0..62]                                   [64, 256]
  q0 = x@Wq + bq ; u = Wk^T q0                           (bk drops: softmax/top-k shift-invariant)
  s_t = (u . tok_t)/16                                   scores row 0
  tokb_m = mean of tok in block m ; qb0 = tokb_0@Wq+bq ; ub = Wk^T qb0
  sblk_m = (ub . tokb_m)/16 ; keep top-2 blocks
  p = softmax(s + mask) ; ctx = sum_t p_t tok_t
  out = relu((x + ctx@(Wv@Wo) + bv@Wo + bo)@W3 + b3)@W4 + b4
"""
import sys
import numpy as np

sys.path.insert(0, '/opt/trn_rl_repo')

import concourse.bass as bass
import concourse.mybir as mybir
import concourse.tile as tile
from concourse import bacc
from concourse.bass_utils import run_bass_kernel_spmd
from concourse.masks import make_identity

F32 = mybir.dt.float32
F32R = mybir.dt.float32r
BF16 = mybir.dt.bfloat16
AL = mybir.AluOpType
AF = mybir.ActivationFunctionType

N_CORES = 8
B = 4096
B_CORE = B // N_CORES          # 512
P = 128                        # partition / b-tile size
N_TILES = B_CORE // P          # 4
OBS = 512
D = 256
HID = 128
NMSG = 63
S = 64                         # tokens = 1 + NMSG
NB = 4                         # blocks
BLK = 16
SCALE = 1.0 / 16.0             # 1/sqrt(D)


def build_kernel(reps=1):
    nc = bacc.Bacc(None, target_bir_lowering=False)

    obs_d = nc.declare_dram_parameter("local_obs", [B_CORE, OBS], F32, isOutput=False)
    msg_d = nc.declare_dram_parameter("messages", [B_CORE, NMSG, D], F32, isOutput=False)
    W1_d = nc.declare_dram_parameter("W1", [OBS, HID], F32R, isOutput=False)
    b1_d = nc.declare_dram_parameter("b1", [HID], F32, isOutput=False)
    W2_d = nc.declare_dram_parameter("W2", [HID, D], F32R, isOutput=False)
    b2_d = nc.declare_dram_parameter("b2", [D], F32, isOutput=False)
    W3_d = nc.declare_dram_parameter("W3", [D, HID], F32R, isOutput=False)
    b3_d = nc.declare_dram_parameter("b3", [HID], F32, isOutput=False)
    W4_d = nc.declare_dram_parameter("W4", [HID, D], F32R, isOutput=False)
    b4_d = nc.declare_dram_parameter("b4", [D], F32, isOutput=False)
    Wq_d = nc.declare_dram_parameter("Wq", [D, D], F32R, isOutput=False)
    bq_d = nc.declare_dram_parameter("bq", [D], F32, isOutput=False)
    Wk_d = nc.declare_dram_parameter("Wk", [D, D], F32R, isOutput=False)
    bk_d = nc.declare_dram_parameter("bk", [D], F32, isOutput=False)  # unused (invariance)
    Wv_d = nc.declare_dram_parameter("Wv", [D, D], F32R, isOutput=False)
    bv_d = nc.declare_dram_parameter("bv", [D], F32R, isOutput=False)
    Wo_d = nc.declare_dram_parameter("Wo", [D, D], F32R, isOutput=False)
    bo_d = nc.declare_dram_parameter("bo", [D], F32, isOutput=False)
    out_d = nc.declare_dram_parameter("out", [B_CORE, D], F32, isOutput=True)

    with tile.TileContext(nc) as tc:
        with tc.tile_pool(name="stat", bufs=1) as stat, \
             tc.tile_pool(name="work", bufs=2) as work, \
             tc.tile_pool(name="gp", bufs=1) as gp, \
             tc.tile_pool(name="loc", bufs=1) as loc, \
             tc.tile_pool(name="psA", bufs=2, space="PSUM") as psA, \
             tc.tile_pool(name="psB", bufs=4, space="PSUM") as psB:

            # ---------------- static setup ----------------
            ident = stat.tile([P, P], F32)
            make_identity(nc, ident[:])

            # enc-path weights first so msg(0) isn't queued behind the rest
            W1s = stat.tile([P, 4, HID], F32R)
            nc.sync.dma_start(out=W1s[:], in_=W1_d.rearrange("(a p) m -> p a m", p=P))
            W2s = stat.tile([P, 1, D], F32R)
            nc.sync.dma_start(out=W2s[:], in_=W2_d.rearrange("(a p) m -> p a m", p=P))
            Wqs = stat.tile([P, 2, D], F32R)
            nc.sync.dma_start(out=Wqs[:], in_=Wq_d.rearrange("(a p) m -> p a m", p=P))
            Wks = stat.tile([P, 2, D], F32R)
            nc.sync.dma_start(out=Wks[:], in_=Wk_d.rearrange("(a p) m -> p a m", p=P))
            b1s = stat.tile([P, 1], F32)
            nc.sync.dma_start(out=b1s[:], in_=b1_d.rearrange("(a p) -> p a", p=P))
            b2s = stat.tile([P, 2], F32)
            nc.sync.dma_start(out=b2s[:], in_=b2_d.rearrange("(a p) -> p a", p=P))
            bqs = stat.tile([P, 2], F32)
            nc.sync.dma_start(out=bqs[:], in_=bq_d.rearrange("(a p) -> p a", p=P))
            WkT = stat.tile([P, 2, D], F32R)
            for it in range(2):
                for nt in range(2):
                    pt = psA.tile([P, P], F32, tag="tpA")
                    nc.tensor.transpose(pt[:], Wks[:, it, nt * P:(nt + 1) * P].bitcast(F32), ident[:])
                    nc.scalar.activation(WkT[:, nt, it * P:(it + 1) * P], pt[:], AF.Identity)

            def setup_late():
                W3s_l = stat.tile([P, 2, HID], F32R, name="W3s")
                nc.sync.dma_start(out=W3s_l[:], in_=W3_d.rearrange("(a p) m -> p a m", p=P))
                W4s_l = stat.tile([P, 1, D], F32R, name="W4s")
                nc.sync.dma_start(out=W4s_l[:], in_=W4_d.rearrange("(a p) m -> p a m", p=P))
                Wvs_l = stat.tile([P, 2, D], F32R, name="Wvs")
                nc.sync.dma_start(out=Wvs_l[:], in_=Wv_d.rearrange("(a p) m -> p a m", p=P))
                Wos_l = stat.tile([P, 2, D], F32R, name="Wos")
                nc.sync.dma_start(out=Wos_l[:], in_=Wo_d.rearrange("(a p) m -> p a m", p=P))
                b3s_l = stat.tile([P, 1], F32, name="b3s")
                nc.sync.dma_start(out=b3s_l[:], in_=b3_d.rearrange("(a p) -> p a", p=P))
                b4rep_l = stat.tile([P, D], F32, name="b4rep")
                nc.gpsimd.dma_start(out=b4rep_l[:], in_=bass.AP(
                    tensor=b4_d.tensor if hasattr(b4_d, 'tensor') else b4_d,
                    offset=0, ap=[[0, P], [1, D]]))
                bvs_l = stat.tile([P, 2], F32R, name="bvs")
                nc.sync.dma_start(out=bvs_l[:], in_=bv_d.rearrange("(a p) -> p a", p=P))
                bos_l = stat.tile([P, 2], F32, name="bos")
                nc.sync.dma_start(out=bos_l[:], in_=bo_d.rearrange("(a p) -> p a", p=P))
                WvT_l = stat.tile([P, 2, D], F32R, name="WvT")
                for it in range(2):
                    for nt in range(2):
                        pt = psA.tile([P, P], F32, tag="tpA")
                        nc.tensor.transpose(pt[:], Wvs_l[:, it, nt * P:(nt + 1) * P].bitcast(F32), ident[:])
                        nc.scalar.activation(WvT_l[:, nt, it * P:(it + 1) * P], pt[:], AF.Identity)
                Wvo_l = stat.tile([P, 2, D], F32R, name="Wvo")
                for it in range(2):
                    pt = psA.tile([P, D], F32, tag="tpA")
                    for nt in range(2):
                        nc.tensor.matmul(pt[:], WvT_l[:, nt, it * P:(it + 1) * P],
                                         Wos_l[:, nt, :], start=(nt == 0), stop=(nt == 1))
                    nc.scalar.activation(Wvo_l[:, it, :], pt[:], AF.Identity)
                bvo_l = stat.tile([P, 2], F32, name="bvo")
                for ot in range(2):
                    pt = psA.tile([P, 1], F32, tag="tpA")
                    for nt in range(2):
                        nc.tensor.matmul(pt[:], Wos_l[:, nt, ot * P:(ot + 1) * P].bitcast(F32),
                                         bvs_l[:, nt:nt + 1].bitcast(F32), start=(nt == 0), stop=(nt == 1))
                    nc.scalar.activation(bvo_l[:, ot:ot + 1], pt[:], AF.Identity)
                nc.vector.tensor_tensor(bvo_l[:], bvo_l[:], bos_l[:], op=AL.add)
                return W3s_l, W4s_l, b3s_l, b4rep_l, Wvo_l, bvo_l

            # ---------------- per-tile pipeline (two stages, interleaved) ----------------
            state = {}

            def stage_a(k, b0):
                st = state[k] = {}
                obs_sb = work.tile([P, OBS], F32, tag="obs", bufs=1)
                nc.sync.dma_start(out=obs_sb[:], in_=obs_d[b0:b0 + P, :])
                msg_a = work.tile([P, 31, D], F32, tag="msga")   # msg rows 0..30  (tok 1..31, blocks 0-1)
                msg_b = work.tile([P, 32, D], F32, tag="msgb")   # msg rows 31..62 (tok 32..63, blocks 2-3)
                nc.sync.dma_start(out=msg_a[:], in_=msg_d[b0:b0 + P, 0:31, :])
                nc.sync.dma_start(out=msg_b[:], in_=msg_d[b0:b0 + P, 31:NMSG, :])

                def msg_row(s):
                    return msg_a[:, s, :] if s < 31 else msg_b[:, s - 31, :]

                # obs_T [128k, 4kt, 128b]
                obs_T = loc.tile([P, 4, P], F32R, tag="obsT")
                for kt in range(4):
                    pt = psA.tile([P, P], F32, tag="tpA")
                    nc.tensor.transpose(pt[:], obs_sb[:, kt * P:(kt + 1) * P], ident[:])
                    nc.scalar.activation(obs_T[:, kt, :], pt[:], AF.Identity)

                # enc1: x1_T = relu(W1^T obs_T + b1)
                x1_T = loc.tile([P, P], F32R, tag="x1T")
                pe1 = psB.tile([P, P], F32, tag="pe")
                for kt in range(4):
                    nc.tensor.matmul(pe1[:], W1s[:, kt, :], obs_T[:, kt, :],
                                     start=(kt == 0), stop=(kt == 3))
                nc.scalar.activation(x1_T[:], pe1[:], AF.Relu, bias=b1s[:])

                # enc2: x_T = relu(W2^T x1_T + b2)
                x_T = work.tile([P, 2, P], F32R, tag="xT", bufs=3)
                st["x_T"] = x_T
                for ot in range(2):
                    pe = psB.tile([P, P], F32, tag="pe")
                    nc.tensor.matmul(pe[:], W2s[:, 0, ot * P:(ot + 1) * P], x1_T[:],
                                     start=True, stop=True)
                    nc.scalar.activation(x_T[:, ot, :], pe[:], AF.Relu, bias=b2s[:, ot:ot + 1])

                # x_b (b-major)
                x_b = work.tile([P, D], F32, tag="xb")
                st["x_b"] = x_b
                for ot in range(2):
                    pt = psA.tile([P, P], F32, tag="tpA")
                    nc.tensor.transpose(pt[:], x_T[:, ot, :].bitcast(F32), ident[:])
                    nc.scalar.activation(x_b[:, ot * P:(ot + 1) * P], pt[:], AF.Identity)

                # q0_T = Wq^T x_T + bq ; u = Wk^T q0 ; u_b (b-major)
                q0_T = loc.tile([P, 2, P], F32R, tag="q0T")
                for ot in range(2):
                    pe = psB.tile([P, P], F32, tag="pe")
                    for kt in range(2):
                        nc.tensor.matmul(pe[:], Wqs[:, kt, ot * P:(ot + 1) * P], x_T[:, kt, :],
                                         start=(kt == 0), stop=(kt == 1))
                    nc.scalar.activation(q0_T[:, ot, :], pe[:], AF.Identity, bias=bqs[:, ot:ot + 1])
                u_b = work.tile([P, D], F32, tag="ub")
                peu = psA.tile([P, D], F32, tag="peu")
                for kt in range(2):
                    nc.tensor.matmul(peu[:], q0_T[:, kt, :], WkT[:, kt, :],
                                     start=(kt == 0), stop=(kt == 1))
                nc.scalar.activation(u_b[:], peu[:], AF.Identity)

                # tokb sums [128b, 4, 256] on GPSIMD (pairwise trees)
                tokb = work.tile([P, NB, D], F32, tag="tokb", bufs=3)
                st["tokb"] = tokb
                g1 = gp.tile([P, 8, D // 8], F32, tag="g1")     # block-1 pairs
                g1b = gp.tile([P, 2, 8, D // 8], F32, tag="g1b") # blocks 2-3 pairs
                g2 = gp.tile([P, 3, 4, D // 8], F32, tag="g2")
                g3 = gp.tile([P, 3, 2, D // 8], F32, tag="g3")
                h1 = gp.tile([P, 8, D // 8], F32, tag="h1")
                h2 = gp.tile([P, 4, D // 8], F32, tag="h2")
                h3 = gp.tile([P, 2, D // 8], F32, tag="h3")
                # block 1: msg rows 15..30 in msg_a ; blocks 2-3: msg rows 31..62 in msg_b
                v1 = msg_a[:, 15:31, :].rearrange("p (i two) d -> p i two d", i=8)
                v23 = msg_b[:].rearrange("p (m i two) d -> p m i two d", m=2, i=8)
                vb = msg_a[:, 0:14, :].rearrange("p (i two) d -> p i two d", i=7)
                for dh in range(8):
                    ds_ = slice(dh * (D // 8), (dh + 1) * (D // 8))
                    nc.gpsimd.tensor_add(g1[:], v1[:, :, 0, ds_], v1[:, :, 1, ds_])
                    nc.gpsimd.tensor_add(g1b[:], v23[:, :, :, 0, ds_], v23[:, :, :, 1, ds_])
                    nc.gpsimd.tensor_add(g2[:, 0, :, :], g1[:, 0::2, :], g1[:, 1::2, :])
                    nc.gpsimd.tensor_add(g2[:, 1:3, :, :], g1b[:, :, 0::2, :], g1b[:, :, 1::2, :])
                    nc.gpsimd.tensor_add(g3[:], g2[:, :, 0::2, :], g2[:, :, 1::2, :])
                    nc.gpsimd.tensor_add(tokb[:, 1:4, ds_], g3[:, :, 0, :], g3[:, :, 1, :])
                    nc.gpsimd.tensor_add(h1[:, 0:7, :], vb[:, :, 0, ds_], vb[:, :, 1, ds_])
                    nc.gpsimd.tensor_add(h1[:, 7, :], msg_a[:, 14, ds_], x_b[:, ds_])
                    nc.gpsimd.tensor_add(h2[:], h1[:, 0::2, :], h1[:, 1::2, :])
                    nc.gpsimd.tensor_add(h3[:], h2[:, 0::2, :], h2[:, 1::2, :])
                    nc.gpsimd.tensor_add(tokb[:, 0, ds_], h3[:, 0, :], h3[:, 1, :])

                # scores s_t = (u . tok_t)/16  [128, 64]  (DVE, fused dots)
                sc = work.tile([P, S], F32, tag="sc")
                dm = loc.tile([P, 1], F32, tag="dm")
                for t in range(1, S):
                    nc.vector.affine_mul_reduce(
                        out=dm.broadcast_to((P, D)), accum_out=sc[:, t:t + 1],
                        in0=msg_row(t - 1), in1=u_b[:], scale=SCALE, bias=0.0)
                nc.vector.affine_mul_reduce(
                    out=dm.broadcast_to((P, D)), accum_out=sc[:, 0:1],
                    in0=x_b[:], in1=u_b[:], scale=SCALE, bias=0.0)

                # unmasked softmax numerators + per-block sums
                mxs = loc.tile([P, 1], F32, tag="mxs")
                nc.vector.tensor_reduce(out=mxs[:], in_=sc[:], axis=mybir.AxisListType.X, op=AL.max)
                nmxs = loc.tile([P, 1], F32, tag="nmxs")
                nc.vector.tensor_scalar(nmxs[:], mxs[:], -1.0, None, op0=AL.mult)
                prob = work.tile([P, S], F32, tag="prob")
                nc.scalar.activation(prob[:], sc[:], AF.Exp, bias=nmxs[:])
                Zh = work.tile([P, NB], F32, tag="Zh", bufs=3)
                st["Zh"] = Zh
                nc.vector.tensor_reduce(
                    out=Zh[:, :, None], in_=prob[:].rearrange("p (nb blk) -> p nb blk", nb=NB),
                    axis=mybir.AxisListType.X, op=AL.add)

                # per-block exp-weighted context sums E_m = sum_{t in m} prob_t tok_t
                st["prob"] = prob
                st["msg_a"] = msg_a
                st["msg_b"] = msg_b

            def stage_a2(k):
                st = state[k]
                prob, msg_a, msg_b, x_b = st["prob"], st["msg_a"], st["msg_b"], st["x_b"]

                def msg_row(s):
                    return msg_a[:, s, :] if s < 31 else msg_b[:, s - 31, :]

                # E_m = sum_{t in m} prob_t tok_t: ACT bf16 products (half-blocks), DVE 2x trees
                E = work.tile([P, NB, D], BF16, tag="E", bufs=3)
                st["E"] = E
                for m in range(NB):
                    eh = []
                    for h in range(2):
                        prod = work.tile([P, 8, D], BF16, tag="prod")
                        for j2 in range(8):
                            t = m * BLK + h * 8 + j2
                            if t == 0:
                                nc.vector.memset(prod[:, 0, :], 0.0)
                            else:
                                nc.scalar.activation(prod[:, j2, :], msg_row(t - 1), AF.Identity,
                                                     scale=prob[:, t:t + 1])
                        l1 = loc.tile([P, 4, D], BF16, tag="l1")
                        nc.vector.tensor_tensor(l1[:], prod[:, 0::2, :], prod[:, 1::2, :], op=AL.add)
                        l2 = loc.tile([P, 2, D], BF16, tag="l2")
                        nc.vector.tensor_tensor(l2[:], l1[:, 0::2, :], l1[:, 1::2, :], op=AL.add)
                        l3 = loc.tile([P, D], BF16, tag="l3" + str(h))
                        nc.vector.tensor_tensor(l3[:], l2[:, 0, :], l2[:, 1, :], op=AL.add)
                        eh.append(l3)
                    nc.vector.tensor_tensor(E[:, m, :], eh[0][:], eh[1][:], op=AL.add)
                nc.vector.scalar_tensor_tensor(
                    out=E[:, 0, :], in0=x_b[:], scalar=prob[:, 0:1],
                    in1=E[:, 0, :], op0=AL.mult, op1=AL.add)


            def stage_b(k, b0):
                st = state.pop(k)
                x_T, x_b, tokb, E, Zh = st["x_T"], st["x_b"], st["tokb"], st["E"], st["Zh"]

                # qb0_T = Wq^T (tokb0/16)_T + bq ; ub = Wk^T qb0 ; ub_b
                tokb0_T = loc.tile([P, 2, P], F32R, tag="tokb0T")
                for ot in range(2):
                    pt = psA.tile([P, P], F32, tag="tpA")
                    nc.tensor.transpose(pt[:], tokb[:, 0, ot * P:(ot + 1) * P], ident[:])
                    nc.scalar.activation(tokb0_T[:, ot, :], pt[:], AF.Identity)
                qb0_T = loc.tile([P, 2, P], F32R, tag="qb0T")
                for ot in range(2):
                    pe = psB.tile([P, P], F32, tag="pe")
                    for kt in range(2):
                        nc.tensor.matmul(pe[:], Wqs[:, kt, ot * P:(ot + 1) * P], tokb0_T[:, kt, :],
                                         start=(kt == 0), stop=(kt == 1))
                    nc.scalar.activation(qb0_T[:, ot, :], pe[:], AF.Identity,
                                         scale=SCALE, bias=bqs[:, ot:ot + 1])
                ub_b = work.tile([P, D], F32, tag="ubb")
                peub = psA.tile([P, D], F32, tag="peu")
                for kt in range(2):
                    nc.tensor.matmul(peub[:], qb0_T[:, kt, :], WkT[:, kt, :],
                                     start=(kt == 0), stop=(kt == 1))
                nc.scalar.activation(ub_b[:], peub[:], AF.Identity)

                # sblk[b, m] = (ub . tokb_m)/16 ; top-2 keep
                sblk = work.tile([P, NB], F32, tag="sblk")
                dm2 = loc.tile([P, 1], F32, tag="dm")
                for m in range(NB):
                    nc.vector.affine_mul_reduce(
                        out=dm2.broadcast_to((P, D)), accum_out=sblk[:, m:m + 1],
                        in0=tokb[:, m, :], in1=ub_b[:], scale=SCALE, bias=0.0)
                mx = loc.tile([P, 1], F32, tag="mx")
                nc.vector.tensor_reduce(out=mx[:], in_=sblk[:], axis=mybir.AxisListType.X, op=AL.max)
                iseq = loc.tile([P, NB], F32, tag="iseq")
                nc.vector.tensor_tensor(iseq[:], sblk[:], mx[:].broadcast_to((P, NB)), op=AL.is_ge)
                masked = loc.tile([P, NB], F32, tag="masked")
                nc.vector.scalar_tensor_tensor(
                    out=masked[:], in0=iseq[:], scalar=-3e38, in1=sblk[:],
                    op0=AL.mult, op1=AL.add)
                thr = loc.tile([P, 1], F32, tag="thr")
                nc.vector.tensor_reduce(out=thr[:], in_=masked[:], axis=mybir.AxisListType.X, op=AL.max)
                keep = work.tile([P, NB], F32, tag="keep")
                nc.vector.tensor_tensor(keep[:], sblk[:], thr[:].broadcast_to((P, NB)), op=AL.is_ge)

                # Z = sum keep_m Zh_m ; ctx = (sum keep_m E_m) / Z
                Z = loc.tile([P, 1], F32, tag="Z")
                dz = loc.tile([P, 1], F32, tag="dz")
                nc.vector.affine_mul_reduce(
                    out=dz.broadcast_to((P, NB)), accum_out=Z[:],
                    in0=Zh[:], in1=keep[:], scale=1.0, bias=0.0)
                rZ = loc.tile([P, 1], F32, tag="rZ")
                nc.vector.reciprocal(rZ[:], Z[:])
                ctx = loc.tile([P, D], F32, tag="ctx")
                nc.vector.tensor_scalar(ctx[:], E[:, 0, :], keep[:, 0:1], None, op0=AL.mult)
                for m in range(1, NB):
                    nc.vector.scalar_tensor_tensor(
                        out=ctx[:], in0=E[:, m, :], scalar=keep[:, m:m + 1],
                        in1=ctx[:], op0=AL.mult, op1=AL.add)
                nc.vector.tensor_scalar(ctx[:], ctx[:], rZ[:], None, op0=AL.mult)

                # ctx_T ; comm0_T = Wvo^T ctx_T ; y_T = comm0 + bvo + x_T
                ctx_T = loc.tile([P, 2, P], F32R, tag="ctxT")
                for ot in range(2):
                    pt = psA.tile([P, P], F32, tag="tpA")
                    nc.tensor.transpose(pt[:], ctx[:, ot * P:(ot + 1) * P], ident[:])
                    nc.scalar.activation(ctx_T[:, ot, :], pt[:], AF.Identity)
                y_T = loc.tile([P, 2, P], F32R, tag="yT")
                for ot in range(2):
                    pe = psB.tile([P, P], F32, tag="pe")
                    for kt in range(2):
                        nc.tensor.matmul(pe[:], Wvo[:, kt, ot * P:(ot + 1) * P], ctx_T[:, kt, :],
                                         start=(kt == 0), stop=(kt == 1))
                    nc.vector.scalar_tensor_tensor(
                        out=y_T[:, ot, :], in0=pe[:], scalar=bvo[:, ot:ot + 1],
                        in1=x_T[:, ot, :].bitcast(F32), op0=AL.add, op1=AL.add)

                # dec1 + dec2 + output transpose + store
                h_T = loc.tile([P, P], F32R, tag="hT")
                ped = psB.tile([P, P], F32, tag="pe")
                for kt in range(2):
                    nc.tensor.matmul(ped[:], W3s[:, kt, :], y_T[:, kt, :],
                                     start=(kt == 0), stop=(kt == 1))
                nc.scalar.activation(h_T[:], ped[:], AF.Relu, bias=b3s[:])
                out_b = loc.tile([P, D], F32, tag="outb")
                peo = psA.tile([P, D], F32, tag="peu")
                nc.tensor.matmul(peo[:], h_T[:], W4s[:, 0, :], start=True, stop=True)
                nc.vector.tensor_tensor(out_b[:], peo[:], b4rep[:], op=AL.add)
                nc.sync.dma_start(out=out_d[b0:b0 + P, :], in_=out_b[:])

            # software-pipelined emission: A1(k) one tile ahead of A2(k); B trails
            tiles = [t for _ in range(reps) for t in range(N_TILES)]
            n = len(tiles)
            order = []
            for k in range(n):
                order.append(("A1", k))
                if k >= 1:
                    order.append(("A2", k - 1))
                if k >= 2:
                    order.append(("B", k - 2))
            order += [("A2", n - 1), ("B", n - 2), ("B", n - 1)]
            W3s = W4s = b3s = b4rep = Wvo = bvo = None
            for oi, (kind, k) in enumerate(order):
                if kind == "A1":
                    stage_a(k, tiles[k] * P)
                elif kind == "A2":
                    stage_a2(k)
                else:
                    stage_b(k, tiles[k] * P)
                if oi == 0:
                    W3s, W4s, b3s, b4rep, Wvo, bvo = setup_late()

    nc.compile()
    return nc


_NC_CACHE = None


def kernel(**inputs):
    global _NC_CACHE
    if _NC_CACHE is None:
        _NC_CACHE = build_kernel()
    nc = _NC_CACHE

    weights = {k: np.ascontiguousarray(v, dtype=np.float32) for k, v in inputs.items()
               if k not in ("local_obs", "messages")}
    obs = np.ascontiguousarray(inputs["local_obs"], dtype=np.float32)
    msg = np.ascontiguousarray(inputs["messages"], dtype=np.float32)

    in_maps = []
    for c in range(N_CORES):
        m = dict(weights)
        m["local_obs"] = obs[c * B_CORE:(c + 1) * B_CORE]
        m["messages"] = msg[c * B_CORE:(c + 1) * B_CORE]
        in_maps.append(m)

    res = run_bass_kernel_spmd(nc, in_maps, list(range(N_CORES))).results
    return np.concatenate([r["out"] for r in res], axis=0)

